# revision 1
# baseline (speedup 1.0000x reference)
"""Trainium2 Bass kernel for nn_Attention_35905926595471.

Channel-attention (XCA-style) block, data-parallel over batch: 8 samples on
8 NeuronCores. Per core: FiLM folded into qkv weights, qkv 1x1 conv on PE
(bf16), 3x3 depthwise conv on DVE (tensor_scalar products at 4x + tensor_tensor
adds at 2x in a zero-padded 130-stride row layout), PE transposes + per-head
Gram in PSUM, norms from Gram diagonal, softmax smalls, attention folded into
the output projection (W2T = A_bd.T @ w_proj.T on PE), final y matmul.
"""
import numpy as np
from contextlib import ExitStack

import concourse.bacc as bacc
import concourse.bass as bass
import concourse.mybir as mybir
from concourse import tile
from concourse.bass_utils import run_bass_kernel_spmd

F32 = mybir.dt.float32
BF16 = mybir.dt.bfloat16
NPBF16 = mybir.dt.np(BF16)

DIM, HEADS, H, W = 192, 6, 128, 128
HD = DIM // HEADS          # 32
N = H * W                  # 16384
NCHUNKS = 8
CH = N // NCHUNKS          # 2048 px (16 rows) per chunk
ROWS = 16
SW = W + 2                 # padded row stride 130
STG = (ROWS + 2) * SW      # stage tile free size 2340
OT = 5
ACT_TAPS = (0, 2, 6, 8)    # tap products offloaded from DVE to ScalarE
AX = mybir.AxisListType
AF = mybir.ActivationFunctionType


def _perm():
    perm = []
    for t in range(3):
        for h in (2 * t, 2 * t + 1):
            perm += list(range(h * HD, (h + 1) * HD))
            perm += list(range(DIM + h * HD, DIM + (h + 1) * HD))
    perm += list(range(2 * DIM, 3 * DIM))
    return np.array(perm)


def _emit(nc, t):
    with ExitStack() as ctx:
        tc = ctx.enter_context(tile.TileContext(nc))
        sb = ctx.enter_context(tc.tile_pool(name="sb", bufs=1))
        stp = ctx.enter_context(tc.tile_pool(name="stage", bufs=4))
        plp = ctx.enter_context(tc.tile_pool(name="plane", bufs=2))
        pla = ctx.enter_context(tc.tile_pool(name="planeact", bufs=2))
        qko = ctx.enter_context(tc.tile_pool(name="qkout", bufs=3))
        qkt = ctx.enter_context(tc.tile_pool(name="qkt", bufs=2))
        ysp = ctx.enter_context(tc.tile_pool(name="ystage", bufs=1))
        pmm = ctx.enter_context(tc.tile_pool(name="pmm", bufs=2, space=bass.MemorySpace.PSUM))
        ptr = ctx.enter_context(tc.tile_pool(name="ptr", bufs=2, space=bass.MemorySpace.PSUM))
        pgr = ctx.enter_context(tc.tile_pool(name="pgr", bufs=1, space=bass.MemorySpace.PSUM))

        # ---- resident tensors (explicit tags: sb pool has bufs=1) ----
        xs = sb.tile([128, 2 * N], BF16, tag="xs", name="xs")     # ch0-127 | ch128-191 on parts 0-63 at +N
        vsb = sb.tile([128, 2 * N], BF16, tag="vsb", name="vsb")        # v ch0-127 | ch128-191 on parts 0-63 at +N
        wkT = [sb.tile([128, 384], F32, tag=f"wkT{i}", name=f"wkT{i}") for i in range(2)]
        wqraw = [sb.tile([128, 640], F32, tag="wqr0", name="wqr0"), sb.tile([64, 640], F32, tag="wqr1", name="wqr1")]
        wqbf = [sb.tile([128, 640], BF16, tag="wqb0", name="wqb0"), sb.tile([64, 640], BF16, tag="wqb1", name="wqb1")]
        wpT = [sb.tile([128, DIM], F32, tag="wpT0", name="wpT0"), sb.tile([64, DIM], F32, tag="wpT1", name="wpT1")]
        wdw = sb.tile([128, OT * 9], F32, tag="wdw", name="wdw")
        idb = sb.tile([128, 128], BF16, tag="idb", name="idb")
        idf = sb.tile([128, 128], F32, tag="idf", name="idf")
        tmpc = sb.tile([128, 3], F32, tag="tmpc", name="tmpc")
        onesr = sb.tile([1, 128], F32, tag="onesr", name="onesr")
        Lsb = [sb.tile([128, 128], F32, tag=f"L{g}", name=f"L{g}") for g in range(3)]
        Asb = [sb.tile([128, DIM], F32, tag="A0", name="A0"), sb.tile([64, DIM], F32, tag="A1", name="A1")]
        dscr = sb.tile([128, 128], F32, tag="dscr", name="dscr")         # diag scratch
        w2t = [sb.tile([128, DIM], BF16, tag="w2t0", name="w2t0"), sb.tile([64, DIM], BF16, tag="w2t1", name="w2t1")]
        sm = sb.tile([128, 16], F32, tag="sm", name="sm")
        nrow = [sb.tile([1, 128], F32, tag=f"nrow{g}", name=f"nrow{g}") for g in range(3)]

        for i in range(2):
            nc.sync.dma_start(wkT[i][:], t["wkT"].ap()[i * 128:(i + 1) * 128, :])
        nc.sync.dma_start(wqraw[0][:], t["wqT"].ap()[0:128, :])
        nc.sync.dma_start(wqraw[1][:], t["wqT"].ap()[128:192, :])
        nc.sync.dma_start(wpT[0][:], t["wpT"].ap()[0:128, :])
        nc.sync.dma_start(wpT[1][:], t["wpT"].ap()[128:192, :])
        nc.sync.dma_start(wdw[:], t["wdw"].ap()[:, :])
        nc.sync.dma_start(idb[:], t["idb"].ap()[:, :])
        nc.sync.dma_start(idf[:], t["idf"].ap()[:, :])
        nc.sync.dma_start(tmpc[:], t["tmpc"].ap()[:, :])
        nc.sync.dma_start(onesr[:], t["onesr"].ap()[:, :])
        for ci in range(NCHUNKS):
            nc.sync.dma_start(xs[:, ci * CH:(ci + 1) * CH], t["xa"].ap()[:, ci * CH:(ci + 1) * CH])
            nc.sync.dma_start(xs[0:64, N + ci * CH:N + (ci + 1) * CH], t["xb"].ap()[:, ci * CH:(ci + 1) * CH])

        # ---- FiLM ----
        kvin = [sb.tile([128, 1], F32, tag=f"kv{i}", name=f"kv{i}") for i in range(2)]
        nc.sync.dma_start(kvin[0][:], t["kv"].ap()[0:128, :])
        nc.sync.dma_start(kvin[1][:], t["kv"].ap()[128:256, :])
        # sm cols: 0=s0,1=s1,2=t0,3=t1,4..8=bias per otile, 9..15 scratch
        for mi, (c0, cn) in enumerate([(0, 128), (128, 64), (192, 128), (320, 64)]):
            ps = ptr.tile([128, 192], F32, tag="pt", name="pt")
            nc.tensor.matmul(ps[0:cn, 0:1], wkT[0][:, c0:c0 + cn], kvin[0][:], start=True, stop=False)
            nc.tensor.matmul(ps[0:cn, 0:1], wkT[1][:, c0:c0 + cn], kvin[1][:], start=False, stop=True)
            nc.scalar.copy(sm[0:cn, mi:mi + 1], ps[0:cn, 0:1])

        nc.vector.tensor_scalar_mul(wqbf[0][:], wqraw[0][:], sm[0:128, 0:1])
        nc.vector.tensor_scalar_mul(wqbf[1][:], wqraw[1][:], sm[0:64, 1:2])
        for ot in range(OT):
            ps = ptr.tile([128, 192], F32, tag="pt", name="pt")
            nc.tensor.matmul(ps[:, 0:1], wqraw[0][:, ot * 128:(ot + 1) * 128], sm[0:128, 2:3], start=True, stop=False)
            nc.tensor.matmul(ps[:, 0:1], wqraw[1][:, ot * 128:(ot + 1) * 128], sm[0:64, 3:4], start=False, stop=True)
            nc.scalar.copy(sm[:, 4 + ot:5 + ot], ps[:, 0:1])

        # ---- main loop ----
        def emit_evict(ot, ci, stages):
            st = stages[ci]
            s3 = st[:].rearrange("p (r c) -> p r c", c=SW)
            if ci == NCHUNKS - 1:
                nc.gpsimd.memset(s3[:, ROWS + 1:ROWS + 2, :], 0.0)
            bias = sm[:, 4 + ot:5 + ot]
            for half in range(2):
                px0 = ci * CH + half * 1024
                ps = pmm.tile([128, 1024], F32, tag="mm", name="mm")
                for q in range(2):
                    q0 = q * 512
                    nc.tensor.matmul(ps[:, q0:q0 + 512], wqbf[0][:, ot * 128:(ot + 1) * 128],
                                     xs[:, px0 + q0:px0 + q0 + 512], start=True, stop=False)
                    nc.tensor.matmul(ps[:, q0:q0 + 512], wqbf[1][:, ot * 128:(ot + 1) * 128],
                                     xs[0:64, N + px0 + q0:N + px0 + q0 + 512], start=False, stop=True)
                p3 = ps[:].rearrange("p (r c) -> p r c", c=W)
                r0 = 1 + half * 8
                nc.scalar.activation(s3[:, r0:r0 + 8, 1:129], p3[:], AF.Identity, bias=bias, scale=1.0)
                if half == 0 and ci > 0:
                    pr3 = stages[ci - 1][:].rearrange("p (r c) -> p r c", c=SW)
                    nc.scalar.activation(pr3[:, ROWS + 1:ROWS + 2, 1:129], p3[:, 0:1, :], AF.Identity, bias=bias, scale=1.0)
                if half == 1 and ci + 1 < NCHUNKS:
                    n3 = stages[ci + 1][:].rearrange("p (r c) -> p r c", c=SW)
                    nc.scalar.activation(n3[:, 0:1, 1:129], p3[:, 7:8, :], AF.Identity, bias=bias, scale=1.0)

        def emit_dw(ot, ci, stages, gram):
            is_v = ot >= 3
            npart = 64 if ot == 4 else 128
            st = stages[ci]
            s3 = st[:].rearrange("p (r c) -> p r c", c=SW)
            if ot == 3:
                acc = vsb[0:128, ci * CH:(ci + 1) * CH]
            elif ot == 4:
                acc = vsb[0:64, N + ci * CH:N + (ci + 1) * CH]
            else:
                qo = qko.tile([128, CH], BF16, tag="qk", name="qk")
                acc = qo[0:128, :]
            a3 = acc.rearrange("p (r c) -> p r c", c=W)
            def wcol(tap):
                return wdw[0:npart, ot * 9 + tap:ot * 9 + tap + 1]
            nc.vector.tensor_scalar_mul(a3, s3[0:npart, 1:1 + ROWS, 1:129], wcol(4))
            for tap in (0, 1, 2, 3, 5, 6, 7, 8):
                dy, dx = tap // 3 - 1, tap % 3 - 1
                if tap in ACT_TAPS:
                    pl = pla.tile([128, STG], BF16, tag="pla", name="pla")
                    nc.scalar.mul(pl[0:npart, :], st[0:npart, :], wcol(tap))
                else:
                    pl = plp.tile([128, STG], BF16, tag="pl", name="pl")
                    nc.vector.tensor_scalar_mul(pl[0:npart, :], st[0:npart, :], wcol(tap))
                pl3 = pl[:].rearrange("p (r c) -> p r c", c=SW)
                nc.vector.tensor_add(a3, a3, pl3[0:npart, 1 + dy:1 + dy + ROWS, 1 + dx:1 + dx + W])
            if not is_v:
                for b2 in range(2):
                    pt = ptr.tile([128, 1024], BF16, tag="pt", name="pt")
                    for b in range(8):
                        blk = b2 * 8 + b
                        nc.tensor.transpose(pt[:, b * 128:(b + 1) * 128], acc[:, blk * 128:(blk + 1) * 128], idb[:])
                    qt = qkt.tile([128, 1024], BF16, tag="qt", name="qt")
                    nc.scalar.copy(qt[:], pt[:])
                    for b in range(8):
                        first = (ci == 0 and b2 == 0 and b == 0)
                        last = (ci == NCHUNKS - 1 and b2 == 1 and b == 7)
                        nc.tensor.matmul(gram[:], qt[:, b * 128:(b + 1) * 128], qt[:, b * 128:(b + 1) * 128],
                                         start=first, stop=last)

        for ot in range(OT):
            is_v = ot >= 3
            gram = None if is_v else pgr.tile([128, 128], F32, tag="gram", name="gram")
            stages = [None] * NCHUNKS

            def new_stage(ci):
                stages[ci] = stp.tile([128, STG], BF16, tag="st", name="st")
                z3 = stages[ci][:].rearrange("p (r c) -> p r c", c=SW)
                nc.gpsimd.memset(z3[:, :, 0:1], 0.0)
                nc.gpsimd.memset(z3[:, :, 129:130], 0.0)
                if ci == 0:
                    nc.gpsimd.memset(z3[:, 0:1, :], 0.0)

            new_stage(0)
            for ci in range(NCHUNKS):
                if ci + 1 < NCHUNKS:
                    new_stage(ci + 1)
                emit_evict(ot, ci, stages)
                if ci >= 1:
                    emit_dw(ot, ci - 1, stages, gram)
            emit_dw(ot, NCHUNKS - 1, stages, gram)
            if not is_v:
                nc.scalar.copy(Lsb[ot][:], gram[:])

        # ---- norms + logits scale + softmax ----
        for g in range(3):
            L = Lsb[g]
            dcol = sm[:, 9:10]
            scr = sm[:, 10:11]
            dsc = sm[:, 11:12]
            nc.vector.tensor_mul(dscr[:], L[:], idf[:])
            nc.vector.reduce_sum(dcol, dscr[:], axis=AX.X)
            nc.scalar.sqrt(scr, dcol)
            nc.vector.tensor_scalar_max(scr, scr, 1e-12)
            nc.vector.reciprocal(dsc, scr)
            rs = sm[:, 12:13]
            nc.vector.tensor_mul(rs, dsc, tmpc[:, g:g + 1])
            pt = ptr.tile([128, 192], F32, tag="pt", name="pt")
            nc.tensor.transpose(pt[0:1, 0:128], dsc, idf[:])
            nc.scalar.copy(nrow[g][:], pt[0:1, 0:128])
            pt2 = ptr.tile([128, 192], F32, tag="pt", name="pt")
            nc.tensor.matmul(pt2[:, 0:128], onesr[:], nrow[g][:], start=True, stop=True)
            nc.vector.tensor_scalar_mul(L[:], L[:], rs)
            nc.vector.tensor_mul(L[:], L[:], pt2[:, 0:128])
            for j in range(2):
                P0, K0 = 64 * j, 64 * j + 32
                mx = sm[P0:P0 + 32, 14:15]
                nc.vector.reduce_max(mx, L[P0:P0 + 32, K0:K0 + 32], axis=AX.X)
                nc.vector.tensor_scalar_sub(L[P0:P0 + 32, K0:K0 + 32], L[P0:P0 + 32, K0:K0 + 32], mx)
                nc.scalar.activation(L[P0:P0 + 32, K0:K0 + 32], L[P0:P0 + 32, K0:K0 + 32], AF.Exp)
                nc.vector.reduce_sum(mx, L[P0:P0 + 32, K0:K0 + 32], axis=AX.X)
                nc.vector.reciprocal(mx, mx)
                nc.vector.tensor_scalar_mul(L[P0:P0 + 32, K0:K0 + 32], L[P0:P0 + 32, K0:K0 + 32], mx)

        # ---- A_bd ----
        nc.gpsimd.memset(Asb[0][:], 0.0)
        nc.gpsimd.memset(Asb[1][:], 0.0)
        for h in range(HEADS):
            g, j = h // 2, h % 2
            src = Lsb[g][64 * j:64 * j + 32, 64 * j + 32:64 * j + 64]
            dst_t = Asb[0] if h < 4 else Asb[1]
            dp = 32 * (h % 4)
            dst = dst_t[dp:dp + 32, 32 * h:32 * h + 32]
            if dp == 64 * j:
                nc.vector.tensor_copy(dst, src)
            else:
                nc.sync.dma_start(dst, src)

        # ---- W2T = A_bd.T @ wpT ----
        for dt_ in range(2):
            c0, cn = dt_ * 128, (128 if dt_ == 0 else 64)
            ps = ptr.tile([128, 192], F32, tag="pt", name="pt")
            nc.tensor.matmul(ps[0:cn, :], Asb[0][:, c0:c0 + cn], wpT[0][:], start=True, stop=False)
            nc.tensor.matmul(ps[0:cn, :], Asb[1][:, c0:c0 + cn], wpT[1][:], start=False, stop=True)
            nc.scalar.copy(w2t[dt_][0:cn, :], ps[0:cn, :])

        # ---- y = W2T.T @ v ----
        for ci in range(16):
            px0 = ci * 1024
            for oT, (o0, on) in enumerate([(0, 128), (128, 64)]):
                ps = pmm.tile([128, 1024], F32, tag="mm", name="mm")
                for q in range(2):
                    q0 = q * 512
                    nc.tensor.matmul(ps[0:on, q0:q0 + 512], w2t[0][:, o0:o0 + on],
                                     vsb[0:128, px0 + q0:px0 + q0 + 512], start=True, stop=False)
                    nc.tensor.matmul(ps[0:on, q0:q0 + 512], w2t[1][:, o0:o0 + on],
                                     vsb[0:64, N + px0 + q0:N + px0 + q0 + 512], start=False, stop=True)
                ys = ysp.tile([128, 1024], F32, tag="ys", name="ys")
                nc.scalar.copy(ys[0:on, :], ps[0:on, :])
                dst = t["yA"] if oT == 0 else t["yB"]
                nc.sync.dma_start(dst.ap()[:, px0:px0 + 1024], ys[0:on, :])


_CACHE = {}


def _module():
    if "nc" in _CACHE:
        return _CACHE["nc"], _CACHE["t"]
    nc = bacc.Bacc("TRN2", target_bir_lowering=False, debug=False)
    t = {
        "xa": nc.dram_tensor("xa", [128, N], BF16, kind="ExternalInput"),
        "xb": nc.dram_tensor("xb", [64, N], BF16, kind="ExternalInput"),
        "kv": nc.dram_tensor("kv", [256, 1], F32, kind="ExternalInput"),
        "wkT": nc.dram_tensor("wkT", [256, 384], F32, kind="ExternalInput"),
        "wqT": nc.dram_tensor("wqT", [192, 640], F32, kind="ExternalInput"),
        "wdw": nc.dram_tensor("wdw", [128, OT * 9], F32, kind="ExternalInput"),
        "wpT": nc.dram_tensor("wpT", [192, DIM], F32, kind="ExternalInput"),
        "idb": nc.dram_tensor("idb", [128, 128], BF16, kind="ExternalInput"),
        "idf": nc.dram_tensor("idf", [128, 128], F32, kind="ExternalInput"),
        "tmpc": nc.dram_tensor("tmpc", [128, 3], F32, kind="ExternalInput"),
        "onesr": nc.dram_tensor("onesr", [1, 128], F32, kind="ExternalInput"),
        "yA": nc.dram_tensor("yA", [128, N], F32, kind="ExternalOutput"),
        "yB": nc.dram_tensor("yB", [64, N], F32, kind="ExternalOutput"),
    }
    _emit(nc, t)
    nc.compile()
    _CACHE["nc"], _CACHE["t"] = nc, t
    return nc, t


def kernel(x, k_v, w_kernel, w_qkv, w_dw, w_proj, temperature):
    x = np.asarray(x, np.float32)
    k_v = np.asarray(k_v, np.float32)
    w_kernel = np.asarray(w_kernel, np.float32)
    w_qkv = np.asarray(w_qkv, np.float32)
    w_dw = np.asarray(w_dw, np.float32)
    w_proj = np.asarray(w_proj, np.float32)
    temperature = np.asarray(temperature, np.float32).reshape(-1)

    perm = _perm()
    wqT = np.zeros((192, 640), np.float32)
    wqT[:, :576] = w_qkv[perm].T
    wdw_p = np.zeros((640, 9), np.float32)
    wdw_p[:576] = w_dw.reshape(3 * DIM, 9)[perm]
    wdw_t = np.zeros((128, OT * 9), np.float32)
    for ot in range(OT):
        wdw_t[:, ot * 9:(ot + 1) * 9] = wdw_p[ot * 128:(ot + 1) * 128]
    wkT = np.ascontiguousarray(w_kernel.T)
    wpT = np.ascontiguousarray(w_proj.T)
    idb = np.eye(128, dtype=NPBF16)
    idf = np.eye(128, dtype=np.float32)
    tmpc = np.ones((128, 3), np.float32)
    for g in range(3):
        for j in range(2):
            tmpc[64 * j:64 * j + 32, g] = temperature[2 * g + j]
    onesr = np.ones((1, 128), np.float32)

    nc, t = _module()
    rep = dict(wkT=wkT, wqT=wqT, wdw=wdw_t, wpT=wpT, idb=idb, idf=idf,
               tmpc=tmpc, onesr=onesr)
    in_maps = []
    for b in range(8):
        xb_ = x[b].reshape(DIM, N)
        m = {"xa": np.ascontiguousarray(xb_[:128]).astype(NPBF16),
             "xb": np.ascontiguousarray(xb_[128:]).astype(NPBF16),
             "kv": np.ascontiguousarray(k_v[b].reshape(256, 1))}
        m.update(rep)
        in_maps.append(m)
    res = run_bass_kernel_spmd(nc, in_maps, core_ids=list(range(8)))
    outs = []
    for b in range(8):
        yA = np.asarray(res.results[b]["yA"])
        yB = np.asarray(res.results[b]["yB"])
        outs.append(np.concatenate([yA, yB], axis=0).reshape(DIM, H, W))
    return np.stack(outs).astype(np.float32)

